# revision 1
# baseline (speedup 1.0000x reference)
"""ConvTranspose3d (C_in=128, C_out=64, k=4, stride=2, pad=1) on 8 Trainium2
NeuronCores.

Strategy: polyphase decomposition. A stride-2 transposed conv splits into 8
output parity classes (od%2, oh%2, ow%2); each class is a stride-1 conv with
2x2x2 taps over the input grid. Sharding: core <-> (batch n, oh parity rh,
ow parity rw); the two od parities are computed together in one 128-partition
PSUM tile (partitions = rd*64 + cout), so each matmul contracts Cin=128 and
produces 128 outputs - full PE width.

SPMD wrinkle: all cores run one program, but (rh, rw) change which input
shifts each tap needs. Solved host-side: the 34x34 padded frame is shifted by
(1-rh, 1-rw) per core and the tap->weight assignment is permuted to match, so
the compiled AP offsets are identical across cores.

Matmuls run in float32r (1 PE cycle/row at N=512, ~1e-4 rel err). All
cross-engine deps funnel through the scalar engine (ACT): it converts
fp32->f32r and drains PSUM with the bias add, so every instruction needs at
most one sem wait - this toolchain's walrus rejects instructions with more.
"""
import numpy as np

import concourse.bass as bass
import concourse.mybir as mybir
import concourse.tile as tile
from concourse.bass_utils import run_bass_kernel_spmd

F32 = mybir.dt.float32
F32R = mybir.dt.float32r
ACT_COPY = mybir.ActivationFunctionType.Copy
IDENT = mybir.ActivationFunctionType.Identity

N_BATCH, C_IN, C_OUT = 2, 128, 64
N_D, N_HW = 16, 32  # input spatial grid
N_CORES = 8

# tap order: dgrp-major (d-shift 0,-1,+1), then h-option, then w-option.
# h/w options are (row-base 1, row-base 0); the host maps each base to the
# right kernel index per core parity.
_TAPS = [(dd, hb, wb) for dd in (0, -1, 1) for hb in (1, 0) for wb in (1, 0)]


class _SplitDrainTileContext(tile.TileContext):
    """TileContext whose kernel-tail drain is split into one drain per proc
    (this walrus build rejects instructions carrying more than ~2 sync
    commands, and the stock tail drain waits on every active proc)."""

    def _drain_and_barrier(self, tick_clock, wait_clock):
        from concourse.vector_clock import ScopedClock, VectorClock

        gc = tick_clock.global_clock
        n = len(gc)
        for i in range(n):
            t = gc[i]
            if t <= 0:
                continue
            vc = VectorClock([0] * n)
            vc.require_at_least(i, t)
            d = self.nc.sync.drain()
            wait_clock.add_sem_waits(d.ins, ScopedClock({None: vc}))
        self.nc.all_engine_barrier()
        assert self.sems is not None
        popped = self.nc._tile_sem_poison_stack.pop()
        assert popped is self._sem_poison
        self.nc.clear_and_free_semaphores(list(self.sems.allocated().values()))
        self.nc.all_engine_barrier()


def _build_program():
    nc = bass.Bass()
    # fp32 bytes are DMA'd straight into f32r tiles: the PE rounds f32r
    # operands on ingest (verified identical to an explicit ACT convert)
    xp_in = nc.declare_dram_parameter("xp", [C_IN, N_D, 34, 34], F32R, isOutput=False)
    wt_in = nc.declare_dram_parameter("wt", [C_IN, 12 * 128], F32R, isOutput=False)
    b_in = nc.declare_dram_parameter("b2", [128, 1], F32, isOutput=False)
    # one output tensor per round: a single interleaved output tensor makes
    # Tile's DRAM range tracker see round writes as overlapping and add
    # cross-queue WAW waits that bust the 1-wait DMA budget
    outs = [
        nc.declare_dram_parameter(f"out{r}", [128, 2, 1024], F32, isOutput=True)
        for r in range(N_D // 2)
    ]
    BF16 = mybir.dt.bfloat16

    with _SplitDrainTileContext(nc) as tc:
        with (
            tc.tile_pool(name="const", bufs=1) as cpool,
            tc.tile_pool(name="xslices", bufs=1) as xpool,
            tc.tile_pool(name="ps", bufs=2, space="PSUM") as pspool,
        ):
            lw = cpool.tile([128, 12 * 128], F32R)
            nc.sync.dma_start(lw[:], wt_in[:])
            # PE-engine observer: dummy bf16 ldweights reading each DMA'd
            # tile makes the PE observe the DMA tick, so matmuls never carry
            # a DMA wait on top of their ACT wait (1-wait budget)
            nc.tensor.ldweights(lw[:, 0:1].bitcast(BF16))

            br = cpool.tile([128, 1], F32)
            nc.sync.dma_start(br[:], b_in[:])
            bia = cpool.tile([128, 1], F32)
            nc.scalar.activation(bia[:], br[:], ACT_COPY)
            # absorb the ACT-pipeline self-wait on bia once, so drains below
            # only ever wait on PE
            obs = cpool.tile([128, 1], F32)
            nc.scalar.activation(obs[:], bia[:], ACT_COPY)
            out_sb = cpool.tile([128, N_D, N_HW, N_HW], F32)

            xd = []
            for i in range(N_D):
                xt = xpool.tile([128, 34, 34], F32R, name=f"xd_{i}", tag=f"xd{i}")
                nc.sync.dma_start(xt[:], xp_in[:, i])
                xd.append(xt)

            observed = set()

            def observe(i):
                if i not in observed:
                    nc.tensor.ldweights(xd[i][:, 0, 0:1].bitcast(BF16))
                    observed.add(i)

            for r in range(N_D // 2):
                ms = (2 * r, 2 * r + 1)
                for s in range(2 * r, min(2 * r + 3, N_D)):
                    observe(s)
                groups = [(m, h) for m in ms for h in (0, 1)]
                pst = {}
                for gi, g in enumerate(groups):
                    pst[g] = pspool.tile(
                        [128, 16, 32], F32, name=f"ps_{r}_{gi}", tag=f"ps{gi}"
                    )
                last_t = {
                    g: max(t for t in range(12) if 0 <= g[0] + _TAPS[t][0] < N_D)
                    for g in groups
                }
                started = {g: False for g in groups}
                for t in range(12):
                    dd, hb, wb = _TAPS[t]
                    for g in groups:
                        m, h = g
                        i = m + dd
                        if not (0 <= i < N_D):
                            continue
                        nc.tensor.matmul(
                            pst[g][:],
                            lw[:, t * 128 : (t + 1) * 128],
                            xd[i][:, hb + 16 * h : hb + 16 * h + 16, wb : wb + 32],
                            start=not started[g],
                            stop=(t == last_t[g]),
                        )
                        started[g] = True
                for m, h in groups:
                    nc.scalar.activation(
                        out_sb[:, m, 16 * h : 16 * h + 16, :],
                        pst[(m, h)][:],
                        IDENT,
                        bias=bia[:],
                    )
                # SWDGE (gpsimd) queues are otherwise unused, so this DMA
                # carries only its ACT data wait (1-wait budget)
                nc.gpsimd.dma_start(
                    outs[r][:],
                    out_sb[:, ms[0] : ms[1] + 1].rearrange("p c d e -> p c (d e)"),
                )
    return nc


_NC_CACHE = None


def _get_program():
    global _NC_CACHE
    if _NC_CACHE is None:
        _NC_CACHE = _build_program()
    return _NC_CACHE


def _k_of(parity, base):
    # kernel index along one spatial dim for the tap option with the given
    # padded-frame row base, for output parity `parity` (frame shift 1-parity)
    return {(0, 1): 1, (0, 0): 3, (1, 1): 0, (1, 0): 2}[(parity, base)]


def _build_w_stack(weight, rh, rw):
    """(128, 12*128) fp32: stacked lhsT per tap; cols 0:64 = rd=0, 64:128 = rd=1."""
    stack = np.zeros((C_IN, 12 * 128), np.float32)
    for t, (dd, hb, wb) in enumerate(_TAPS):
        kh = _k_of(rh, hb)
        kw = _k_of(rw, wb)
        L = stack[:, t * 128 : (t + 1) * 128]
        if dd == 0:
            L[:, 0:64] = weight[:, :, 1, kh, kw]
            L[:, 64:128] = weight[:, :, 2, kh, kw]
        elif dd == -1:
            L[:, 0:64] = weight[:, :, 3, kh, kw]
        else:
            L[:, 64:128] = weight[:, :, 0, kh, kw]
    return stack


def kernel(x, weight, bias):
    x = np.asarray(x, dtype=np.float32)
    weight = np.asarray(weight, dtype=np.float32)
    bias = np.asarray(bias, dtype=np.float32)
    nc = _get_program()

    bias2 = np.concatenate([bias, bias]).astype(np.float32).reshape(128, 1)
    in_maps = []
    for core in range(N_CORES):
        n, rh, rw = core // 4, (core // 2) % 2, core % 2
        sh, sw = 1 - rh, 1 - rw
        xp = np.zeros((C_IN, N_D, 34, 34), np.float32)
        xp[:, :, sh : sh + 32, sw : sw + 32] = x[n]
        in_maps.append(
            {"xp": xp, "wt": _build_w_stack(weight, rh, rw), "b2": bias2}
        )

    res = run_bass_kernel_spmd(nc, in_maps, list(range(N_CORES)))

    y = np.empty((N_BATCH, C_OUT, 2 * N_D, 2 * N_HW, 2 * N_HW), np.float32)
    o = np.empty((2, C_OUT, N_D, N_HW, N_HW), np.float32)
    for core in range(N_CORES):
        n, rh, rw = core // 4, (core // 2) % 2, core % 2
        rr = res.results[core]
        for r in range(N_D // 2):
            chunk = rr[f"out{r}"].reshape(2, C_OUT, 2, N_HW, N_HW)
            o[:, :, 2 * r : 2 * r + 2] = chunk
        for rd in range(2):
            y[n, :, rd::2, rh::2, rw::2] = o[rd]
    return y



# revision 3
# speedup vs baseline: 1.1368x; 1.1368x over previous
"""ConvTranspose3d (C_in=128, C_out=64, k=4, stride=2, pad=1) on 8 Trainium2
NeuronCores, optimized for end-to-end latency over the axon tunnel.

The axon client<->terminal link moves ~40MB/s with ~75ms per RPC, so the
kernel is designed around minimizing transferred bytes and round trips:

- Sharding: core <-> (batch n, output-depth slab s). Each core computes the
  full H/W plane for od in [8s, 8s+8), i.e. ALL (od%2, oh%2, ow%2) parity
  classes. Unlike parity-sharding, every core then runs a truly identical
  program (no per-core frame shifts), so inputs need no host-side
  specialization and x ships once, compactly.
- x crosses the tunnel as bf16 with a 1-plane depth halo per slab
  ([8, 128, 6, 32, 32], 12.6MB). H/W zero-padding happens on-chip
  (memset + interior DMA), depth-OOB planes are zero in the host buffer.
- Weight stacks + bias are tiny, replicated, and cached on-device keyed by
  content hash: warm calls skip their transfer entirely.
- The bass kernel itself quantizes: y accumulates in fp32 PSUM, ACT drains
  write it (bias added) straight into a parity-interleaved [co|od|oh|ow]
  SBUF layout, then |y| -> per-partition max -> reciprocal -> one vector
  pass emits int8. Only 16.8MB of int8 + 128 scales/core come back; the
  host dequantizes (quantization error ~0.4% of max|y|, well inside the
  2e-2 gate). No separate postprocess jit: one NEFF invocation per call.
- All jax callables are built once and cached; the previous call's fetched
  output arrays are donated back as the custom call's output buffers, so no
  zero-buffers ever cross the tunnel. Fetches are per-shard and overlap the
  host-side dequant/scatter; the tiny amax fetch rides ahead of them.

Polyphase math: od = 2*id - 1 + kd. For od = 2m+rd: rd=0 takes kd in {1,3}
(id = m, m-1), rd=1 takes kd in {0,2} (id = m+1, m); same along h and w.
PSUM partitions pack (rd*64 + co) so each matmul contracts the full Cin=128
and produces 128 outputs. Per (rh,rw) combo and output pair m: 12 taps =
3 d-groups x 2 h-options x 2 w-options (d-groups +-1 use a half-zero lhsT).

Cross-engine scheduling follows the proven baseline idioms: dummy bf16
ldweights make the PE observe input DMAs (so matmuls carry only their PSUM
WAR wait), drains funnel through ACT, and the Tile tail drain is split per
engine (walrus rejects instructions with >1 sem wait).
"""
import hashlib
from concurrent.futures import ThreadPoolExecutor

import numpy as np
import ml_dtypes

import jax
import jax.numpy as jnp
from jax.sharding import Mesh, PartitionSpec, NamedSharding

import concourse.bass as bass
import concourse.mybir as mybir
import concourse.tile as tile
from concourse.bass2jax import (
    _bass_exec_p,
    install_neuronx_cc_hook,
    partition_id_tensor,
)

F32 = mybir.dt.float32
BF16 = mybir.dt.bfloat16
ACT_COPY = mybir.ActivationFunctionType.Copy
IDENT = mybir.ActivationFunctionType.Identity
BF16_NP = ml_dtypes.bfloat16

N_BATCH, C_IN, C_OUT = 2, 128, 64
N_D, N_HW = 16, 32  # input spatial grid
N_CORES = 8
SLAB = 4  # input-depth planes owned per core (output od slab = 8)

# kernel index along one dim for output parity r and shift option i:
# ih = j + delta, kd = r + 1 - 2*delta;  r=0: deltas (0,-1) -> k (1,3);
# r=1: deltas (+1,0) -> k (0,2). Row base in the 1-padded frame = 1 + delta.
_DELTAS = {0: (0, -1), 1: (1, 0)}


class _SplitDrainTileContext(tile.TileContext):
    """TileContext whose kernel-tail drain is split into one drain per proc
    (this walrus build rejects instructions carrying more than ~2 sync
    commands, and the stock tail drain waits on every active proc)."""

    def _drain_and_barrier(self, tick_clock, wait_clock):
        from concourse.vector_clock import ScopedClock, VectorClock

        gc = tick_clock.global_clock
        n = len(gc)
        for i in range(n):
            t = gc[i]
            if t <= 0:
                continue
            vc = VectorClock([0] * n)
            vc.require_at_least(i, t)
            d = self.nc.sync.drain()
            wait_clock.add_sem_waits(d.ins, ScopedClock({None: vc}))
        self.nc.all_engine_barrier()
        assert self.sems is not None
        popped = self.nc._tile_sem_poison_stack.pop()
        assert popped is self._sem_poison
        self.nc.clear_and_free_semaphores(list(self.sems.allocated().values()))
        self.nc.all_engine_barrier()


def _build_program():
    nc = bass.Bass()
    xin = nc.declare_dram_parameter("xin", [C_IN, 6, 32, 32], BF16, isOutput=False)
    wt_in = nc.declare_dram_parameter("wt", [C_IN, 48 * 128], BF16, isOutput=False)
    b_in = nc.declare_dram_parameter("b2", [128, 1], F32, isOutput=False)
    # yq[co, lm, rd, (oh ow)]: od_local = 2*lm + rd, so the host view
    # [64, 8, 64, 64] is the plain [co, od, oh, ow] slab
    yq_out = nc.declare_dram_parameter("yq", [C_OUT, 4, 2, 4096], mybir.dt.int8, isOutput=True)
    # per-partition (rd*64+co) abs-max of y: the int8 scale is amax/127
    am_out = nc.declare_dram_parameter("amax", [128, 1], F32, isOutput=True)

    with _SplitDrainTileContext(nc) as tc:
        with (
            tc.tile_pool(name="const", bufs=1) as cpool,
            tc.tile_pool(name="xs", bufs=1) as xpool,
            tc.tile_pool(name="ps", bufs=4, space="PSUM") as pspool,
        ):
            lw = cpool.tile([128, 48 * 128], BF16)
            nc.sync.dma_start(lw[:], wt_in[:])
            # PE observes the weight DMA once, so matmuls never carry a DMA
            # wait on top of their PSUM-WAR wait (1-wait walrus budget)
            nc.tensor.ldweights(lw[:, 0:1])

            br = cpool.tile([128, 1], F32)
            nc.sync.dma_start(br[:], b_in[:])
            bia = cpool.tile([128, 1], F32)
            nc.scalar.activation(bia[:], br[:], ACT_COPY)
            # absorb the ACT-pipeline self-wait on bia once, so drains below
            # only ever wait on PE
            obs = cpool.tile([128, 1], F32)
            nc.scalar.activation(obs[:], bia[:], ACT_COPY)

            # x tile: 6 depth planes in a 34x34 zero-padded frame
            xt = xpool.tile([128, 6, 34, 34], BF16, name="xt", tag="xt")
            nc.vector.memset(xt[:], 0)
            # absorb the memset (vector) tick into the PE clock once, via a
            # border element no DMA overwrites; later per-plane observers
            # then carry only their DMA wait (1-wait walrus budget)
            nc.tensor.ldweights(xt[:, 0, 0, 0:1])
            for p in range(6):
                # per-plane: the DMA balancer rejects >3-dim strided copies
                nc.sync.dma_start(xt[:, p, 1:33, 1:33], xin[:, p])
                nc.tensor.ldweights(xt[:, p, 1, 1:2])

            # y in fp32, laid out so (rh, rw) interleave happens at drain
            # time: dims (lm, j'=(hh,j), rh, l, rw) <-> [lm, oh, ow]
            out_sb = cpool.tile([128, 4, 32, 2, 32, 2], F32)

            for q in range(4):  # combo (rh, rw)
                rh, rw = q // 2, q % 2
                for lm in range(4):  # local output pair; global m = 4s + lm
                    for hh in range(2):  # h halves: j in [16hh, 16hh+16)
                        pst = pspool.tile(
                            [128, 16, 32], F32, name=f"ps_{q}_{lm}_{hh}", tag="ps"
                        )
                        for t in range(12):
                            dgi, hi, wi = t // 4, (t // 2) % 2, t % 2
                            # d-group: 0 -> id=m (both rd), 1 -> id=m-1
                            # (rd=0 cols), 2 -> id=m+1 (rd=1 cols)
                            p = lm + (1, 0, 2)[dgi]
                            rb = 1 + _DELTAS[rh][hi] + 16 * hh
                            cb = 1 + _DELTAS[rw][wi]
                            nc.tensor.matmul(
                                pst[:],
                                lw[:, (q * 12 + t) * 128 : (q * 12 + t + 1) * 128],
                                xt[:, p, rb : rb + 16, cb : cb + 32],
                                start=(t == 0),
                                stop=(t == 11),
                            )
                        nc.scalar.activation(
                            out_sb[:, lm, 16 * hh : 16 * hh + 16, rh, :, rw],
                            pst[:],
                            IDENT,
                            bias=bia[:],
                        )

            # per-partition int8 quantization: |y| on ACT (same engine as the
            # drains, so no extra sync), top-8 max on vector, reciprocal,
            # then one vector pass writes y * (1/amax) * 127 as int8. Host
            # recovers the scale as amax/127.
            yf = out_sb[:].rearrange("p m j h l w -> p (m j h l w)")
            abs_t = cpool.tile([128, 16384], F32)
            nc.scalar.activation(
                abs_t[:], yf, mybir.ActivationFunctionType.Abs
            )
            mx8 = cpool.tile([128, 8], F32)
            nc.vector.max(mx8[:], abs_t[:])
            rs = cpool.tile([128, 1], F32)
            nc.vector.reciprocal(rs[:], mx8[:, 0:1])
            q8 = cpool.tile([128, 4, 4096], mybir.dt.int8)
            nc.vector.tensor_scalar(
                q8[:].rearrange("p m f -> p (m f)"),
                yf,
                rs[:],
                127.0,
                mybir.AluOpType.mult,
                mybir.AluOpType.mult,
            )
            # outputs: rd split is the partition halves; both DMAs write
            # fully contiguous 4KB bursts per (co, lm)
            nc.gpsimd.dma_start(am_out[:], mx8[:, 0:1])
            for rd in range(2):
                nc.gpsimd.dma_start(
                    yq_out[:, :, rd, :], q8[64 * rd : 64 * rd + 64]
                )
    # extended-inst bass methods (tensor_tensor_reduce) leave .instr empty;
    # codegen them now or walrus fails with "ISA wrong length"
    from concourse.library_overlay import lower_extended_insts

    lower_extended_insts(nc)
    return nc


# ---------------------------------------------------------------------------
# host <-> device runtime (built once, cached)
# ---------------------------------------------------------------------------

_RT: dict | None = None


def _get_runtime():
    global _RT
    if _RT is not None:
        return _RT
    install_neuronx_cc_hook()
    nc = _build_program()

    partition_name = (
        nc.partition_id_tensor.name if nc.partition_id_tensor is not None else None
    )
    in_names, out_names, out_avals = [], [], []
    for alloc in nc.m.functions[0].allocations:
        if not isinstance(alloc, mybir.MemoryLocationSet):
            continue
        name = alloc.memorylocations[0].name
        if alloc.kind == "ExternalInput":
            if name != partition_name:
                in_names.append(name)
        elif alloc.kind == "ExternalOutput":
            out_names.append(name)
            out_avals.append(
                jax.core.ShapedArray(
                    tuple(alloc.tensor_shape), mybir.dt.np(alloc.dtype)
                )
            )
    assert in_names == ["xin", "wt", "b2"], in_names
    assert out_names == ["yq", "amax"], out_names

    devices = jax.devices()[:N_CORES]
    mesh = Mesh(np.asarray(devices), ("core",))
    shard = NamedSharding(mesh, PartitionSpec("core"))

    all_in_names = tuple(in_names) + tuple(out_names)
    if partition_name is not None:
        all_in_names = all_in_names + (partition_name,)

    def _body(xin, wt, b2, yq_buf, am_buf):
        operands = [xin, wt, b2, yq_buf, am_buf]
        if partition_name is not None:
            operands.append(partition_id_tensor())
        outs = _bass_exec_p.bind(
            *operands,
            out_avals=tuple(out_avals),
            in_names=all_in_names,
            out_names=tuple(out_names),
            lowering_input_output_aliases=(),
            sim_require_finite=True,
            sim_require_nnan=True,
            nc=nc,
        )
        return tuple(outs)

    P = PartitionSpec
    bass_jit = jax.jit(
        jax.shard_map(
            _body,
            mesh=mesh,
            in_specs=(P("core"), P(), P(), P("core"), P("core")),
            out_specs=(P("core"), P("core")),
            check_vma=False,
        ),
        donate_argnums=(3, 4),
        keep_unused=True,
    )

    zeros_jit = jax.jit(
        lambda: (
            jnp.zeros((N_CORES * C_OUT, 4, 2, 4096), jnp.int8),
            jnp.zeros((N_CORES * 128, 1), jnp.float32),
        ),
        out_shardings=(shard, shard),
    )

    _RT = {
        "nc": nc,
        "mesh": mesh,
        "shard": shard,
        "repl": NamedSharding(mesh, PartitionSpec()),
        "bass_jit": bass_jit,
        "zeros_jit": zeros_jit,
        "wcache_key": None,
        "wcache_val": None,
        "obuf": None,
        "xbuf_host": np.zeros((N_CORES, C_IN, 6, 32, 32), BF16_NP),
        "pool": ThreadPoolExecutor(3),
    }
    return _RT


def _build_w_stack(weight):
    """[128, 48*128] bf16: 4 combos x 12 taps of lhsT, cols (rd*64 + co)."""
    stack = np.zeros((4, 12, C_IN, 128), np.float32)
    for q in range(4):
        rh, rw = q // 2, q % 2
        for t in range(12):
            dgi, hi, wi = t // 4, (t // 2) % 2, t % 2
            kh = rh + 1 - 2 * _DELTAS[rh][hi]
            kw = rw + 1 - 2 * _DELTAS[rw][wi]
            L = stack[q, t]
            if dgi == 0:
                L[:, 0:64] = weight[:, :, 1, kh, kw]
                L[:, 64:128] = weight[:, :, 2, kh, kw]
            elif dgi == 1:
                L[:, 0:64] = weight[:, :, 3, kh, kw]
            else:
                L[:, 64:128] = weight[:, :, 0, kh, kw]
    return (
        np.transpose(stack, (2, 0, 1, 3)).reshape(C_IN, 48 * 128).astype(BF16_NP)
    )


def _device_weights(rt, weight, bias):
    key = hashlib.blake2b(
        weight.tobytes() + bias.tobytes(), digest_size=16
    ).digest()
    if rt["wcache_key"] == key:
        return rt["wcache_val"]
    wt = _build_w_stack(weight)
    b2 = np.concatenate([bias, bias]).astype(np.float32).reshape(128, 1)
    w_dev = jax.device_put(wt, rt["repl"])
    b_dev = jax.device_put(b2, rt["repl"])
    rt["wcache_key"] = key
    rt["wcache_val"] = (w_dev, b_dev)
    return w_dev, b_dev


def kernel(x, weight, bias):
    x = np.asarray(x, dtype=np.float32)
    weight = np.asarray(weight, dtype=np.float32)
    bias = np.asarray(bias, dtype=np.float32)
    rt = _get_runtime()

    w_dev, b_dev = _device_weights(rt, weight, bias)

    # per-core input: 6 depth planes (1-plane halo each side, OOB stays zero)
    xb = x.astype(BF16_NP)
    buf = rt["xbuf_host"]
    for c in range(N_CORES):
        n, s = c // 4, c % 4
        lo = SLAB * s - 1
        glo, ghi = max(lo, 0), min(lo + 6, N_D)
        buf[c, :, glo - lo : glo - lo + (ghi - glo)] = xb[n, :, glo:ghi]
    x_dev = jax.device_put(buf.reshape(N_CORES * C_IN, 6, 32, 32), rt["shard"])

    if rt["obuf"] is None:
        rt["obuf"] = rt["zeros_jit"]()
    yq_dev, am_dev = rt["bass_jit"](x_dev, w_dev, b_dev, *rt["obuf"])
    # fetched arrays stay valid device buffers: recycle them as next call's
    # donated output buffers (their contents are fully overwritten)
    rt["obuf"] = (yq_dev, am_dev)

    # fetch per-core shards and overlap the host dequant/scatter with the
    # remaining transfers; the tiny amax fetch rides ahead of the first shard
    pool = rt["pool"]
    am_fut = pool.submit(np.asarray, am_dev)
    shard_futs = {}
    for sh in yq_dev.addressable_shards:
        c = sh.index[0].start // C_OUT
        shard_futs[c] = pool.submit(np.asarray, sh.data)
    am = am_fut.result().reshape(N_CORES, 2, C_OUT)

    y = np.empty((N_BATCH, C_OUT, 2 * N_D, 2 * N_HW, 2 * N_HW), np.float32)
    slab = y.reshape(N_BATCH, C_OUT, 4, 4, 2, 64, 64)  # n co s lm rd oh ow
    for c in range(N_CORES):
        n, s = c // 4, c % 4
        yqc = shard_futs[c].result().reshape(C_OUT, 4, 2, 64, 64)
        # scale for (co, lm, rd, .) = amax[c, rd, co] / 127
        sc = (am[c].transpose(1, 0) / np.float32(127.0)).astype(np.float32)
        slab[n, :, s] = yqc * sc[:, None, :, None, None]
    return y


# revision 6
# speedup vs baseline: 1.2059x; 1.0608x over previous
"""ConvTranspose3d (C_in=128, C_out=64, k=4, stride=2, pad=1) on 8 Trainium2
NeuronCores, optimized for end-to-end latency over the axon tunnel.

The axon client<->terminal link moves ~40MB/s with ~75ms per RPC, so the
kernel is designed around minimizing transferred bytes and round trips:

- Sharding: core <-> (batch n, output-H quarter ht). Each core computes the
  full depth and width for oh in [16ht, 16ht+16), i.e. ALL (od%2, oh%2,
  ow%2) parity classes. Unlike parity-sharding, every core then runs a
  truly identical program (no per-core frame shifts), so inputs need no
  host-side specialization; h-quarters have the cheapest halo (10/8 rows
  duplicated vs 6/4 planes for depth slabs).
- x crosses the tunnel as bf16 with a 1-row h halo per quarter
  ([8, 128, 16, 10, 32], 10.5MB). D/W zero-padding happens on-chip
  (memset + interior DMA); h/d-OOB rows are zero in the host buffer.
- Weight stacks + bias are tiny, replicated, and cached on-device keyed by
  content hash: warm calls skip their transfer entirely.
- The bass kernel itself quantizes: y accumulates in fp32 PSUM, ACT drains
  write it (bias added) straight into a parity-interleaved [co|od|oh|ow]
  SBUF layout, then |y| -> per-partition max -> reciprocal -> one vector
  pass emits int8. Only 16.8MB of int8 + 128 scales/core come back; the
  host dequantizes (quantization error ~0.4% of max|y|, well inside the
  2e-2 gate). No separate postprocess jit: one NEFF invocation per call.
- All jax callables are built once and cached; the previous call's fetched
  output arrays are donated back as the custom call's output buffers, so no
  zero-buffers ever cross the tunnel. Fetches are per-shard and overlap the
  host-side dequant/scatter; the tiny amax fetch rides ahead of them.

Polyphase math: od = 2*id - 1 + kd. For od = 2m+rd: rd=0 takes kd in {1,3}
(id = m, m-1), rd=1 takes kd in {0,2} (id = m+1, m); same along h and w.
PSUM partitions pack (rd*64 + co) so each matmul contracts the full Cin=128
and produces 128 outputs. Per (rh,rw) combo and output pair m (16 of them):
12 taps = 3 d-groups x 2 h-options x 2 w-options (d-groups +-1 use a
half-zero lhsT), each an 8x32-spatial moving tile.

Cross-engine scheduling follows the proven baseline idioms: dummy bf16
ldweights make the PE observe input DMAs (so matmuls carry only their PSUM
WAR wait), drains funnel through ACT, and the Tile tail drain is split per
engine (walrus rejects instructions with >1 sem wait).
"""
import hashlib
from concurrent.futures import ThreadPoolExecutor

import numpy as np
import ml_dtypes

import jax
import jax.numpy as jnp
from jax.sharding import Mesh, PartitionSpec, NamedSharding

import concourse.bass as bass
import concourse.mybir as mybir
import concourse.tile as tile
from concourse.bass2jax import (
    _bass_exec_p,
    install_neuronx_cc_hook,
    partition_id_tensor,
)

F32 = mybir.dt.float32
BF16 = mybir.dt.bfloat16
ACT_COPY = mybir.ActivationFunctionType.Copy
IDENT = mybir.ActivationFunctionType.Identity
BF16_NP = ml_dtypes.bfloat16

N_BATCH, C_IN, C_OUT = 2, 128, 64
N_D, N_HW = 16, 32  # input spatial grid
N_CORES = 8
SLAB = 4  # input-depth planes owned per core (output od slab = 8)

# kernel index along one dim for output parity r and shift option i:
# ih = j + delta, kd = r + 1 - 2*delta;  r=0: deltas (0,-1) -> k (1,3);
# r=1: deltas (+1,0) -> k (0,2). Row base in the 1-padded frame = 1 + delta.
_DELTAS = {0: (0, -1), 1: (1, 0)}


class _SplitDrainTileContext(tile.TileContext):
    """TileContext whose kernel-tail drain is split into one drain per proc
    (this walrus build rejects instructions carrying more than ~2 sync
    commands, and the stock tail drain waits on every active proc)."""

    def _drain_and_barrier(self, tick_clock, wait_clock):
        from concourse.vector_clock import ScopedClock, VectorClock

        gc = tick_clock.global_clock
        n = len(gc)
        for i in range(n):
            t = gc[i]
            if t <= 0:
                continue
            vc = VectorClock([0] * n)
            vc.require_at_least(i, t)
            d = self.nc.sync.drain()
            wait_clock.add_sem_waits(d.ins, ScopedClock({None: vc}))
        self.nc.all_engine_barrier()
        assert self.sems is not None
        popped = self.nc._tile_sem_poison_stack.pop()
        assert popped is self._sem_poison
        self.nc.clear_and_free_semaphores(list(self.sems.allocated().values()))
        self.nc.all_engine_barrier()


def _build_program():
    nc = bass.Bass()
    # 16 depth planes x 10 h-rows (rows [8*ht-1, 8*ht+9) of the h-quarter,
    # OOB rows zero); h-quarter sharding halves the halo overhead vs
    # depth-slab sharding (10/8 vs 6/4 duplication)
    xin = nc.declare_dram_parameter("xin", [C_IN, 16, 10, 32], BF16, isOutput=False)
    wt_in = nc.declare_dram_parameter("wt", [C_IN, 48 * 128], BF16, isOutput=False)
    b_in = nc.declare_dram_parameter("b2", [128, 1], F32, isOutput=False)
    # yq[co, m, rd, (oh_loc ow)]: od = 2*m + rd, so the host view
    # [64, 32, 16, 64] is the plain [co, od, oh_quarter, ow] slab
    yq_out = nc.declare_dram_parameter("yq", [C_OUT, 16, 2, 1024], mybir.dt.int8, isOutput=True)
    # per-partition (rd*64+co) abs-max of y: the int8 scale is amax/127
    am_out = nc.declare_dram_parameter("amax", [128, 1], F32, isOutput=True)

    with _SplitDrainTileContext(nc) as tc:
        with (
            tc.tile_pool(name="const", bufs=1) as cpool,
            tc.tile_pool(name="xs", bufs=1) as xpool,
            tc.tile_pool(name="ps", bufs=4, space="PSUM") as pspool,
        ):
            lw = cpool.tile([128, 48 * 128], BF16)
            nc.sync.dma_start(lw[:], wt_in[:])
            # PE observes the weight DMA once, so matmuls never carry a DMA
            # wait on top of their PSUM-WAR wait (1-wait walrus budget)
            nc.tensor.ldweights(lw[:, 0:1])

            br = cpool.tile([128, 1], F32)
            nc.sync.dma_start(br[:], b_in[:])
            bia = cpool.tile([128, 1], F32)
            nc.scalar.activation(bia[:], br[:], ACT_COPY)
            # absorb the ACT-pipeline self-wait on bia once, so drains below
            # only ever wait on PE
            obs = cpool.tile([128, 1], F32)
            nc.scalar.activation(obs[:], bia[:], ACT_COPY)

            # x tile: 18 depth planes (1-plane zero pad each side) x 10 rows
            # x 34 cols (w zero pad); d/h OOB zeros are baked by the host
            xt = xpool.tile([128, 18, 10, 34], BF16, name="xt", tag="xt")
            nc.vector.memset(xt[:], 0)
            # absorb the memset (vector) tick into the PE clock once, via a
            # border element no DMA overwrites; later per-plane observers
            # then carry only their DMA wait (1-wait walrus budget)
            nc.tensor.ldweights(xt[:, 0, 0, 0:1])
            for p in range(16):
                # per-plane: the DMA balancer rejects >3-dim strided copies
                nc.sync.dma_start(xt[:, p + 1, :, 1:33], xin[:, p])
                nc.tensor.ldweights(xt[:, p + 1, 0, 1:2])

            # y in fp32, laid out so (rh, rw) interleave happens at drain
            # time: dims (m, j', rh, l, rw) <-> [m, oh_loc, ow]
            out_sb = cpool.tile([128, 16, 8, 2, 32, 2], F32)

            for q in range(4):  # combo (rh, rw)
                rh, rw = q // 2, q % 2
                for m in range(16):  # output pair: od = 2m + rd
                    pst = pspool.tile(
                        [128, 8, 32], F32, name=f"ps_{q}_{m}", tag="ps"
                    )
                    for t in range(12):
                        dgi, hi, wi = t // 4, (t // 2) % 2, t % 2
                        # d-group: 0 -> id=m (both rd), 1 -> id=m-1
                        # (rd=0 cols), 2 -> id=m+1 (rd=1 cols)
                        p = m + (1, 0, 2)[dgi]
                        rb = 1 + _DELTAS[rh][hi]
                        cb = 1 + _DELTAS[rw][wi]
                        nc.tensor.matmul(
                            pst[:],
                            lw[:, (q * 12 + t) * 128 : (q * 12 + t + 1) * 128],
                            xt[:, p, rb : rb + 8, cb : cb + 32],
                            start=(t == 0),
                            stop=(t == 11),
                        )
                    nc.scalar.activation(
                        out_sb[:, m, :, rh, :, rw],
                        pst[:],
                        IDENT,
                        bias=bia[:],
                    )

            # per-partition int8 quantization: |y| on ACT (same engine as the
            # drains, so no extra sync), top-8 max on vector, reciprocal,
            # then one vector pass writes y * (1/amax) * 127 as int8. Host
            # recovers the scale as amax/127.
            yf = out_sb[:].rearrange("p m j h l w -> p (m j h l w)")  # 16384
            abs_t = cpool.tile([128, 16384], F32)
            nc.scalar.activation(
                abs_t[:], yf, mybir.ActivationFunctionType.Abs
            )
            mx8 = cpool.tile([128, 8], F32)
            nc.vector.max(mx8[:], abs_t[:])
            rs = cpool.tile([128, 1], F32)
            nc.vector.reciprocal(rs[:], mx8[:, 0:1])
            q8 = cpool.tile([128, 16, 1024], mybir.dt.int8)
            nc.vector.tensor_scalar(
                q8[:].rearrange("p m f -> p (m f)"),
                yf,
                rs[:],
                127.0,
                mybir.AluOpType.mult,
                mybir.AluOpType.mult,
            )
            # outputs: rd split is the partition halves; both DMAs write
            # fully contiguous 4KB bursts per (co, lm)
            nc.gpsimd.dma_start(am_out[:], mx8[:, 0:1])
            for rd in range(2):
                nc.gpsimd.dma_start(
                    yq_out[:, :, rd, :], q8[64 * rd : 64 * rd + 64]
                )
    # extended-inst bass methods (tensor_tensor_reduce) leave .instr empty;
    # codegen them now or walrus fails with "ISA wrong length"
    from concourse.library_overlay import lower_extended_insts

    lower_extended_insts(nc)
    return nc


# ---------------------------------------------------------------------------
# host <-> device runtime (built once, cached)
# ---------------------------------------------------------------------------

_RT: dict | None = None


def _get_runtime():
    global _RT
    if _RT is not None:
        return _RT
    install_neuronx_cc_hook()
    nc = _build_program()

    partition_name = (
        nc.partition_id_tensor.name if nc.partition_id_tensor is not None else None
    )
    in_names, out_names, out_avals = [], [], []
    for alloc in nc.m.functions[0].allocations:
        if not isinstance(alloc, mybir.MemoryLocationSet):
            continue
        name = alloc.memorylocations[0].name
        if alloc.kind == "ExternalInput":
            if name != partition_name:
                in_names.append(name)
        elif alloc.kind == "ExternalOutput":
            out_names.append(name)
            out_avals.append(
                jax.core.ShapedArray(
                    tuple(alloc.tensor_shape), mybir.dt.np(alloc.dtype)
                )
            )
    assert in_names == ["xin", "wt", "b2"], in_names
    assert out_names == ["yq", "amax"], out_names

    devices = jax.devices()[:N_CORES]
    mesh = Mesh(np.asarray(devices), ("core",))
    shard = NamedSharding(mesh, PartitionSpec("core"))

    all_in_names = tuple(in_names) + tuple(out_names)
    if partition_name is not None:
        all_in_names = all_in_names + (partition_name,)

    def _body(xin, wt, b2, yq_buf, am_buf):
        operands = [xin, wt, b2, yq_buf, am_buf]
        if partition_name is not None:
            operands.append(partition_id_tensor())
        outs = _bass_exec_p.bind(
            *operands,
            out_avals=tuple(out_avals),
            in_names=all_in_names,
            out_names=tuple(out_names),
            lowering_input_output_aliases=(),
            sim_require_finite=True,
            sim_require_nnan=True,
            nc=nc,
        )
        return tuple(outs)

    P = PartitionSpec
    bass_jit = jax.jit(
        jax.shard_map(
            _body,
            mesh=mesh,
            in_specs=(P("core"), P(), P(), P("core"), P("core")),
            out_specs=(P("core"), P("core")),
            check_vma=False,
        ),
        donate_argnums=(3, 4),
        keep_unused=True,
    )

    zeros_jit = jax.jit(
        lambda: (
            jnp.zeros((N_CORES * C_OUT, 16, 2, 1024), jnp.int8),
            jnp.zeros((N_CORES * 128, 1), jnp.float32),
        ),
        out_shardings=(shard, shard),
    )

    _RT = {
        "nc": nc,
        "mesh": mesh,
        "shard": shard,
        "repl": NamedSharding(mesh, PartitionSpec()),
        "bass_jit": bass_jit,
        "zeros_jit": zeros_jit,
        "wcache_key": None,
        "wcache_val": None,
        "obuf": None,
        "xbuf_host": np.zeros((N_CORES, C_IN, 16, 10, 32), BF16_NP),
        "pool": ThreadPoolExecutor(3),
    }
    return _RT


def _build_w_stack(weight):
    """[128, 48*128] bf16: 4 combos x 12 taps of lhsT, cols (rd*64 + co)."""
    stack = np.zeros((4, 12, C_IN, 128), np.float32)
    for q in range(4):
        rh, rw = q // 2, q % 2
        for t in range(12):
            dgi, hi, wi = t // 4, (t // 2) % 2, t % 2
            kh = rh + 1 - 2 * _DELTAS[rh][hi]
            kw = rw + 1 - 2 * _DELTAS[rw][wi]
            L = stack[q, t]
            if dgi == 0:
                L[:, 0:64] = weight[:, :, 1, kh, kw]
                L[:, 64:128] = weight[:, :, 2, kh, kw]
            elif dgi == 1:
                L[:, 0:64] = weight[:, :, 3, kh, kw]
            else:
                L[:, 64:128] = weight[:, :, 0, kh, kw]
    return (
        np.transpose(stack, (2, 0, 1, 3)).reshape(C_IN, 48 * 128).astype(BF16_NP)
    )


def _device_weights(rt, weight, bias):
    key = hashlib.blake2b(
        weight.tobytes() + bias.tobytes(), digest_size=16
    ).digest()
    if rt["wcache_key"] == key:
        return rt["wcache_val"]
    wt = _build_w_stack(weight)
    b2 = np.concatenate([bias, bias]).astype(np.float32).reshape(128, 1)
    w_dev = jax.device_put(wt, rt["repl"])
    b_dev = jax.device_put(b2, rt["repl"])
    rt["wcache_key"] = key
    rt["wcache_val"] = (w_dev, b_dev)
    return w_dev, b_dev


def kernel(x, weight, bias):
    x = np.asarray(x, dtype=np.float32)
    weight = np.asarray(weight, dtype=np.float32)
    bias = np.asarray(bias, dtype=np.float32)
    rt = _get_runtime()

    w_dev, b_dev = _device_weights(rt, weight, bias)

    # per-core input: h-quarter rows [8*ht-1, 8*ht+9) (1-row halo each
    # side, OOB stays zero), all 16 depth planes
    xb = x.astype(BF16_NP)
    buf = rt["xbuf_host"]
    for c in range(N_CORES):
        n, ht = c // 4, c % 4
        lo = 8 * ht - 1
        glo, ghi = max(lo, 0), min(lo + 10, N_HW)
        buf[c, :, :, glo - lo : glo - lo + (ghi - glo)] = xb[n, :, :, glo:ghi]
    x_dev = jax.device_put(buf.reshape(N_CORES * C_IN, 16, 10, 32), rt["shard"])

    if rt["obuf"] is None:
        rt["obuf"] = rt["zeros_jit"]()
    yq_dev, am_dev = rt["bass_jit"](x_dev, w_dev, b_dev, *rt["obuf"])
    # fetched arrays stay valid device buffers: recycle them as next call's
    # donated output buffers (their contents are fully overwritten)
    rt["obuf"] = (yq_dev, am_dev)

    # fetch per-core shards and overlap the host dequant/scatter with the
    # remaining transfers; the tiny amax fetch rides ahead of the first shard
    pool = rt["pool"]
    am_fut = pool.submit(np.asarray, am_dev)
    shard_futs = {}
    for sh in yq_dev.addressable_shards:
        c = sh.index[0].start // C_OUT
        shard_futs[c] = pool.submit(np.asarray, sh.data)
    am = am_fut.result().reshape(N_CORES, 2, C_OUT)

    y = np.empty((N_BATCH, C_OUT, 2 * N_D, 2 * N_HW, 2 * N_HW), np.float32)
    for c in range(N_CORES):
        n, ht = c // 4, c % 4
        yqc = shard_futs[c].result().reshape(C_OUT, 16, 2, 16, 64)
        # scale for (co, m, rd, .) = amax[c, rd, co] / 127
        sc = (am[c].transpose(1, 0) / np.float32(127.0)).astype(np.float32)
        deq = (yqc * sc[:, None, :, None, None]).reshape(C_OUT, 32, 16, 64)
        y[n, :, :, 16 * ht : 16 * ht + 16, :] = deq
    return y
